# revision 48
# baseline (speedup 1.0000x reference)
"""Multi-head cross-attention on 8 Trainium2 NeuronCores.  (~250 us HW)

Sharding: core = (batch b, T-half). Each core computes the full output slab
out[b, t0:t0+512, :] locally: q projection for its rows, k/v projection for
its batch (duplicated across the 2 cores sharing a batch), attention for all
16 heads, and the output projection. No collectives.

Per-core dataflow (projections fp32r, attention core bf16):
  - PE-transpose x_slab -> xT [C, T], y -> yT [C, S]  (fp32r, via identity)
  - qT[hd, T]  = Wq_p.T chunks @ xT          (fp32r, weights stationary)
  - kT pair j  -> kt0/kt1 [128, S] bf16, the sibling head's 64 rows ZEROED:
                 scores then run as full-grid K=128 matmuls (no tile_position,
                 background weight buffer + FWL stay enabled -> b2b pitch)
  - scoresT + exp(scale*s) are emitted inside phase B right after each kT
    chunk so the ACT engine's 95 us of exp work hides under the qkv matmuls
    (no max-subtraction needed: |s*scale| < ~2, exp in [e^-2, e^2])
  - v[S, h, 65] bf16 (col 64 = ones -> AV row 64 = softmax denominators)
  - av[65, T] = [v_h | 1].T @ expT           (bf16, full-grid)
  - denominators parked at partition bases 0/32/64/96 of one tile -> one
    batched DVE reciprocal per 4 heads; 1/den broadcast via K=1 ones-matmul;
    at = avs * bcast (DVE)
  - out[T, C] = AT.T-chunks @ WoT + ones.T @ bo   (fp32r; bias via K=1 matmul)
"""

import numpy as np
from contextlib import ExitStack

import concourse.bass as bass
import concourse.bacc as bacc
import concourse.mybir as mybir
import concourse.tile as tile
from concourse.bass_utils import run_bass_kernel_spmd

F32 = mybir.dt.float32
F32R = mybir.dt.float32r
BF16 = mybir.dt.bfloat16
EXP = mybir.ActivationFunctionType.Exp
COPY = mybir.ActivationFunctionType.Copy

# Full problem constants (nn_MultiHeadCrossAttention: B,T,S,C,H,D)
B_FULL, T_FULL, S_FULL, C_FULL, H_FULL, D_FULL = 4, 1024, 1024, 1024, 16, 64
N_CORES = 8


def build_attention_nc(T=512, S=1024, C=1024, H=16, D=64, scale=None,
                       et_bufs=44):
    """Per-core kernel. T rows of queries, S source rows, all H heads."""
    assert T % 128 == 0 and S % 128 == 0 and C % 128 == 0 and D == 64
    assert T <= 512  # scores/AV moving-operand free size (one PSUM bank)
    if scale is None:
        scale = C ** (-0.5)
    HD = H * D
    CK, SK, TK, MK = C // 128, S // 128, T // 128, HD // 128

    def nchunks(total):
        return [(i, min(512, total - i)) for i in range(0, total, 512)]

    nc = bacc.Bacc("TRN2", target_bir_lowering=False, debug=False,
                   num_devices=N_CORES)
    x = nc.dram_tensor("x", [T, C], F32R, kind="ExternalInput")
    y = nc.dram_tensor("y", [S, C], F32R, kind="ExternalInput")
    wq = nc.dram_tensor("wq", [C, HD], F32R, kind="ExternalInput")
    wk = nc.dram_tensor("wk", [C, HD], F32R, kind="ExternalInput")
    wv = nc.dram_tensor("wv", [C, HD], F32R, kind="ExternalInput")
    wot = nc.dram_tensor("wot", [HD, C], F32R, kind="ExternalInput")
    bo = nc.dram_tensor("bo", [1, C], F32R, kind="ExternalInput")
    ident_dr = nc.dram_tensor("ident", [128, 128], F32R, kind="ExternalInput")
    ones_dr = nc.dram_tensor("onesc", [128, 128], F32R, kind="ExternalInput")
    o = nc.dram_tensor("o", [T, C], F32, kind="ExternalOutput")

    def wpool_alloc(stack, poolname, rows, cols, side=None):
        """Open a bufs=1 pool holding `rows//128` weight tiles [128, cols]."""
        p = stack.enter_context(tc.tile_pool(name=poolname, bufs=1, side=side))
        return [p.tile([128, cols], F32R, tag=f"{poolname}{k}",
                       name=f"{poolname}{k}") for k in range(rows // 128)]

    def wdma(tiles, dram):
        for k, t in enumerate(tiles):
            nc.sync.dma_start(out=t, in_=dram[k * 128:(k + 1) * 128, :])

    def wpool_load(stack, poolname, dram, rows, cols, side=None):
        tiles = wpool_alloc(stack, poolname, rows, cols, side=side)
        wdma(tiles, dram)
        return tiles

    # SBUF pools are a two-sided stack allocator: open/close must be LIFO
    # per side.  Left: pers > yt > (wq > xt > stage), wk, (at, et).
    # Right: wv, then (wot > dpool).
    with tile.TileContext(nc) as tc, ExitStack() as ctx:
        pers = ctx.enter_context(tc.tile_pool(name="pers", bufs=1))

        ident = pers.tile([128, 128], F32R, tag="ident")
        nc.sync.dma_start(out=ident[:], in_=ident_dr[:, :])
        ones_sb = pers.tile([1, 128], F32R, tag="ones")
        nc.sync.dma_start(out=ones_sb[:], in_=ones_dr[0:1, :])

        qt = [pers.tile([128, T], BF16, tag=f"qt{m}", name=f"qt{m}")
              for m in range(MK)]
        kt0 = [pers.tile([128, S], BF16, tag=f"kt0_{m}", name=f"kt0_{m}")
               for m in range(MK)]
        kt1 = [pers.tile([128, S], BF16, tag=f"kt1_{m}", name=f"kt1_{m}")
               for m in range(MK)]
        v_sb = [pers.tile([128, H, 65], BF16, tag=f"v{s}", name=f"v{s}")
                for s in range(SK)]

        # explicit phase-scoped stacks (closed LIFO per side)
        s_wq, s_wk, s_wv, s_wot = ExitStack(), ExitStack(), ExitStack(), ExitStack()
        s_stage, s_xt, s_yt = ExitStack(), ExitStack(), ExitStack()
        s_psa, s_psb, s_psc, s_pss = (ExitStack(), ExitStack(),
                                      ExitStack(), ExitStack())
        try:
            # ---- phase A: load + transpose x, y ---------------------------
            p_yt = s_yt.enter_context(tc.tile_pool(name="ytp", bufs=1))
            yt = [p_yt.tile([128, S], F32R, tag=f"yt{k}", name=f"ytt{k}")
                  for k in range(CK)]

            wq_sb = wpool_alloc(s_wq, "wq", C, HD)

            p_xt = s_xt.enter_context(tc.tile_pool(name="xtp", bufs=1))
            xt = [p_xt.tile([128, T], F32R, tag=f"xt{k}", name=f"xtt{k}")
                  for k in range(CK)]

            p_stage = s_stage.enter_context(tc.tile_pool(name="stage", bufs=4))
            ps_t = s_psa.enter_context(
                tc.tile_pool(name="ps_t", bufs=4, space="PSUM"))

            def load_transpose(dram, n_tiles, i0, dst_tiles, dst_off):
                """Load up to 4 row-tiles of one C-half of `dram`; transpose
                the 4 C-chunks of that half into dst tiles."""
                n = min(4, n_tiles - i0)
                for coff, csz in nchunks(C):
                    stages = []
                    for i in range(n):
                        st = p_stage.tile([128, csz], F32R, tag="stage",
                                          name="stage")
                        nc.sync.dma_start(
                            out=st,
                            in_=dram[(i0 + i) * 128:(i0 + i + 1) * 128,
                                     coff:coff + csz])
                        stages.append(st)
                    for q2 in range(csz // 128):
                        ck = coff // 128 + q2
                        pst = ps_t.tile([128, n * 128], F32R, tag="pst",
                                        name="pst")
                        for q in range(n):
                            # pure permutation, no accumulation happens
                            with nc.allow_low_precision(reason="transpose"):
                                nc.tensor.transpose(
                                    pst[:, q * 128:(q + 1) * 128],
                                    stages[q][:, q2 * 128:(q2 + 1) * 128],
                                    ident[:])
                        nc.vector.tensor_copy(
                            dst_tiles[ck][:, dst_off:dst_off + n * 128],
                            pst[:])

            for i0 in range(0, TK, 4):
                load_transpose(x, TK, i0, xt, i0 * 128)
            wdma(wq_sb, wq)   # after x stages: x transposes start promptly
            for i0 in range(0, SK, 4):
                load_transpose(y, SK, i0, yt, i0 * 128)
            s_stage.close()
            s_psa.close()

            # ---- phase B: qT, kT ------------------------------------------
            ps_s = s_pss.enter_context(
                tc.tile_pool(name="ps_s", bufs=4, space="PSUM"))
            ps_qk = s_psb.enter_context(
                tc.tile_pool(name="ps_qk", bufs=3, space="PSUM"))
            for m in range(MK):
                psq = ps_qk.tile([128, T], F32, tag="psqk", name="psq")
                for k in range(CK):
                    nc.tensor.matmul(
                        psq[:], wq_sb[k][:, m * 128:(m + 1) * 128],
                        xt[k][:], start=(k == 0), stop=(k == CK - 1))
                nc.vector.tensor_copy(qt[m][:], psq[:])
            s_xt.close()
            s_wq.close()

            # et pool opens on the right BEFORE wv so scores/exp can start
            # in phase B while ACT is otherwise idle (LIFO: et outlives wv).
            p_et = ctx.enter_context(
                tc.tile_pool(name="etpool", bufs=et_bufs, side="right"))
            ets = {}

            def emit_scores(j):
                """scoresT + exp for head pair j (full-grid K=128 matmuls
                against zero-padded kt0/kt1)."""
                et0, et1 = [], []
                for s in range(SK):
                    pss0 = ps_s.tile([128, T], F32, tag="pss", name="pss")
                    nc.tensor.matmul(
                        pss0[:], kt0[j][:, s * 128:(s + 1) * 128],
                        qt[j][:, :], start=True, stop=True)
                    pss1 = ps_s.tile([128, T], F32, tag="pss", name="pss")
                    nc.tensor.matmul(
                        pss1[:], kt1[j][:, s * 128:(s + 1) * 128],
                        qt[j][:, :], start=True, stop=True)
                    e0 = p_et.tile([128, T], BF16, tag="et", name="et")
                    nc.scalar.activation(out=e0[:], in_=pss0[:], func=EXP,
                                         scale=float(scale))
                    e1 = p_et.tile([128, T], BF16, tag="et", name="et")
                    nc.scalar.activation(out=e1[:], in_=pss1[:], func=EXP,
                                         scale=float(scale))
                    et0.append(e0)
                    et1.append(e1)
                ets[j] = (et0, et1)

            wk_sb = wpool_load(s_wk, "wk", wk, C, HD)
            wv_sb = wpool_load(s_wv, "wv", wv, C, HD, side="right")

            for m in range(MK):
                nc.vector.memset(kt0[m][64:128, :], 0.0)
                nc.vector.memset(kt1[m][0:64, :], 0.0)
                for off, sz in nchunks(S):
                    psk = ps_qk.tile([128, sz], F32, tag="psqk", name="psk")
                    for k in range(CK):
                        nc.tensor.matmul(
                            psk[:], wk_sb[k][:, m * 128:(m + 1) * 128],
                            yt[k][:, off:off + sz],
                            start=(k == 0), stop=(k == CK - 1))
                    nc.vector.tensor_copy(kt0[m][0:64, off:off + sz],
                                          psk[0:64, :])
                    nc.vector.tensor_copy(kt1[m][64:128, off:off + sz],
                                          psk[64:128, :])
                emit_scores(m)
            s_wk.close()
            s_psb.close()

            # ---- phase C: v natural + ones column -------------------------
            ps_v = s_psc.enter_context(
                tc.tile_pool(name="ps_v", bufs=3, space="PSUM"))
            for s in range(SK):
                nc.vector.memset(v_sb[s][:, :, 64:65], 1.0)
            for off, sz in nchunks(HD):
                for s in range(SK):
                    psv = ps_v.tile([128, sz], F32, tag="psv", name="psv")
                    for k in range(CK):
                        nc.tensor.matmul(
                            psv[:], yt[k][:, s * 128:(s + 1) * 128],
                            wv_sb[k][:, off:off + sz],
                            start=(k == 0), stop=(k == CK - 1))
                    h0 = off // 64
                    nc.vector.tensor_copy(
                        v_sb[s][:, h0:h0 + sz // 64, 0:64],
                        psv[:].rearrange("p (h d) -> p h d", d=64))
            s_wv.close()
            s_yt.close()
            s_psc.close()

            # ---- phase D: attention per head ------------------------------
            wot_sb = wpool_load(s_wot, "wot", wot, HD, C, side="right")

            with ExitStack() as ctx_d:
                p_d = ctx_d.enter_context(
                    tc.tile_pool(name="dpool", bufs=2, side="right"))
                bo_sb = p_d.tile([1, C], F32R, tag="bo", bufs=1, name="bo_sb")
                nc.sync.dma_start(out=bo_sb, in_=bo[:, :])

                p_at = ctx_d.enter_context(tc.tile_pool(name="atpool", bufs=1))
                at = [p_at.tile([128, T], F32R, tag=f"at{m}", name=f"at{m}")
                      for m in range(MK)]
                ps_av = ctx_d.enter_context(
                    tc.tile_pool(name="ps_av", bufs=2, space="PSUM"))

                # 4 heads' denominators parked at partition bases 0/32/64/96
                # of one tile -> a single batched DVE reciprocal (DVE recip
                # cost is free-size-bound, so 1 op for 4 heads).
                group = []          # [(head, avs_tile)] awaiting normalize
                den4 = [None]

                def flush_group(last=False):
                    if not group:
                        return
                    rec4 = den4[0]
                    nc.vector.reciprocal(rec4[:], rec4[:])   # in place
                    for i, (h, avs) in enumerate(group):
                        r0 = p_d.tile([1, T], F32R, tag="r0", name="r0")
                        nc.vector.tensor_copy(
                            r0[:], rec4[32 * i:32 * i + 1, :])
                        psb = ps_av.tile([64, T], F32, tag="psav",
                                         name="psbt")
                        nc.tensor.matmul(psb[:], ones_sb[0:1, 0:64],
                                         r0[:], start=True, stop=True)
                        nc.vector.tensor_mul(
                            at[h // 2][(h % 2) * 64:(h % 2) * 64 + 64, :],
                            avs[:], psb[:])
                    group.clear()
                    den4[0] = None

                def stage_av(h, psav):
                    """Move AV rows out of PSUM right away; park denom row."""
                    if den4[0] is None:
                        den4[0] = p_d.tile([97, T], F32, tag="den4",
                                           name="den4")
                        nc.vector.memset(den4[0][:], 1.0)
                    avs = p_d.tile([64, T], F32, tag="avs", bufs=6, name="avs")
                    nc.vector.tensor_copy(avs[:], psav[0:64, :])
                    b = 32 * len(group)
                    nc.vector.tensor_copy(den4[0][b:b + 1, :], psav[64:65, :])
                    group.append((h, avs))

                def emit_av(j, et0, et1):
                    """AV for pair j — 16 back-to-back full-row matmuls."""
                    psav0 = ps_av.tile([65, T], F32, tag="psav", name="psav")
                    for s in range(SK):
                        nc.tensor.matmul(psav0[:], v_sb[s][:, 2 * j, 0:65],
                                         et0[s][:],
                                         start=(s == 0), stop=(s == SK - 1))
                    stage_av(2 * j, psav0)
                    psav1 = ps_av.tile([65, T], F32, tag="psav", name="psav")
                    for s in range(SK):
                        nc.tensor.matmul(psav1[:], v_sb[s][:, 2 * j + 1, 0:65],
                                         et1[s][:],
                                         start=(s == 0), stop=(s == SK - 1))
                    stage_av(2 * j + 1, psav1)

                for j in range(H // 2):
                    if len(group) == 4 or (j == H // 2 - 1 and group):
                        flush_group()
                    emit_av(j, *ets[j])
                flush_group(last=True)

                # ---- phase E: output projection + bias --------------------
                ps_o = ctx_d.enter_context(
                    tc.tile_pool(name="ps_o", bufs=2, space="PSUM"))
                for t_ in range(TK):
                    o_sb = p_d.tile([128, C], F32, tag="osb", name="o_sb")
                    for off, sz in nchunks(C):
                        pso = ps_o.tile([128, sz], F32, tag="pso", name="pso")
                        for mk in range(MK):
                            nc.tensor.matmul(
                                pso[:], at[mk][:, t_ * 128:(t_ + 1) * 128],
                                wot_sb[mk][:, off:off + sz],
                                start=(mk == 0), stop=False)
                        nc.tensor.matmul(pso[:], ones_sb[0:1, 0:128],
                                         bo_sb[0:1, off:off + sz],
                                         start=False, stop=True)
                        nc.scalar.activation(out=o_sb[:, off:off + sz],
                                             in_=pso[:], func=COPY)
                    nc.sync.dma_start(out=o[t_ * 128:(t_ + 1) * 128, :],
                                      in_=o_sb[:])
            s_wot.close()
        finally:
            for s in (s_stage, s_xt, s_wq, s_wk, s_psa, s_psb, s_psc,
                      s_wv, s_yt, s_wot, s_pss):
                s.close()

    nc.compile()
    return nc


# ---------------------------------------------------------------------------
# Host-side wrapper
# ---------------------------------------------------------------------------

_NC_CACHE = {}
_IDENT = np.eye(128, dtype=np.float32)
_ONESC = np.ones((128, 128), dtype=np.float32)


def _get_nc():
    key = "full"
    if key not in _NC_CACHE:
        _NC_CACHE[key] = build_attention_nc(
            T=T_FULL * B_FULL // N_CORES, S=S_FULL, C=C_FULL, H=H_FULL,
            D=D_FULL, scale=C_FULL ** (-0.5))
    return _NC_CACHE[key]


def make_in_maps(x, y_enc, Wq, Wk, Wv, Wo, bo):
    """Shard full inputs into the 8 per-core input maps."""
    x = np.asarray(x, dtype=np.float32)
    y_enc = np.asarray(y_enc, dtype=np.float32)
    Wq = np.asarray(Wq, dtype=np.float32)
    Wk = np.asarray(Wk, dtype=np.float32)
    Wv = np.asarray(Wv, dtype=np.float32)
    Wo = np.asarray(Wo, dtype=np.float32)
    bo = np.asarray(bo, dtype=np.float32)

    C = Wq.shape[1]
    HD = Wq.shape[0] * Wq.shape[2]
    wq_p = np.ascontiguousarray(Wq.transpose(1, 0, 2).reshape(C, HD))
    wk_p = np.ascontiguousarray(Wk.transpose(1, 0, 2).reshape(C, HD))
    wv_p = np.ascontiguousarray(Wv.transpose(1, 0, 2).reshape(C, HD))
    wot = np.ascontiguousarray(Wo.T)
    bo2 = np.ascontiguousarray(bo.reshape(1, -1))

    T = x.shape[1] * x.shape[0] // N_CORES  # rows per core
    in_maps = []
    for core in range(N_CORES):
        b, half = divmod(core, N_CORES // x.shape[0])
        in_maps.append({
            "x": np.ascontiguousarray(x[b, half * T:(half + 1) * T]),
            "y": np.ascontiguousarray(y_enc[b]),
            "wq": wq_p, "wk": wk_p, "wv": wv_p, "wot": wot, "bo": bo2,
            "ident": _IDENT, "onesc": _ONESC,
        })
    return in_maps


def run(inputs, trace=False, trace_cores=None):
    """Compile + run on the 8 cores; returns (out, BassKernelResults)."""
    nc = _get_nc()
    in_maps = make_in_maps(**inputs)
    kw = {}
    if trace:
        kw = dict(trace=True,
                  trace_cores=trace_cores if trace_cores is not None else [0])
    res = run_bass_kernel_spmd(nc, in_maps, core_ids=list(range(N_CORES)), **kw)

    B, T_full, C = np.asarray(inputs["x"]).shape
    T = T_full * B // N_CORES
    out = np.empty((B, T_full, C), dtype=np.float32)
    for core in range(N_CORES):
        b, half = divmod(core, N_CORES // B)
        out[b, half * T:(half + 1) * T] = res.results[core]["o"]
    return out, res


def kernel(x, y_enc, Wq, Wk, Wv, Wo, bo):
    out, _ = run(dict(x=x, y_enc=y_enc, Wq=Wq, Wk=Wk, Wv=Wv, Wo=Wo, bo=bo))
    return out


# revision 49
# speedup vs baseline: 1.0194x; 1.0194x over previous
"""Multi-head cross-attention on 8 Trainium2 NeuronCores.  (~250 us HW)

Sharding: core = (batch b, T-half). Each core computes the full output slab
out[b, t0:t0+512, :] locally: q projection for its rows, k/v projection for
its batch (duplicated across the 2 cores sharing a batch), attention for all
16 heads, and the output projection. No collectives.

Per-core dataflow (projections fp32r, attention core bf16):
  - PE-transpose x_slab -> xT [C, T], y -> yT [C, S]  (fp32r, via identity)
  - qT[hd, T]  = Wq_p.T chunks @ xT          (fp32r, weights stationary)
  - kT pair j  -> kt0/kt1 [128, S] bf16, the sibling head's 64 rows ZEROED:
                 scores then run as full-grid K=128 matmuls (no tile_position,
                 background weight buffer + FWL stay enabled -> b2b pitch)
  - scoresT + exp(scale*s) are emitted inside phase B right after each kT
    chunk so the ACT engine's 95 us of exp work hides under the qkv matmuls
    (no max-subtraction needed: |s*scale| < ~2, exp in [e^-2, e^2])
  - v[S, h, 65] bf16 (col 64 = ones -> AV row 64 = softmax denominators)
  - av[65, T] = [v_h | 1].T @ expT           (bf16, full-grid)
  - denominators parked at partition bases 0/32/64/96 of one tile -> one
    batched DVE reciprocal per 4 heads; 1/den broadcast via K=1 ones-matmul;
    at = avs * bcast (DVE)
  - out[T, C] = AT.T-chunks @ WoT + ones.T @ bo   (fp32r; bias via K=1 matmul)
"""

import numpy as np
from contextlib import ExitStack

import concourse.bass as bass
import concourse.bacc as bacc
import concourse.mybir as mybir
import concourse.tile as tile
from concourse.bass_utils import run_bass_kernel_spmd

F32 = mybir.dt.float32
F32R = mybir.dt.float32r
BF16 = mybir.dt.bfloat16
EXP = mybir.ActivationFunctionType.Exp
COPY = mybir.ActivationFunctionType.Copy

# Full problem constants (nn_MultiHeadCrossAttention: B,T,S,C,H,D)
B_FULL, T_FULL, S_FULL, C_FULL, H_FULL, D_FULL = 4, 1024, 1024, 1024, 16, 64
N_CORES = 8


def build_attention_nc(T=512, S=1024, C=1024, H=16, D=64, scale=None,
                       et_bufs=36):
    """Per-core kernel. T rows of queries, S source rows, all H heads."""
    assert T % 128 == 0 and S % 128 == 0 and C % 128 == 0 and D == 64
    assert T <= 512  # scores/AV moving-operand free size (one PSUM bank)
    if scale is None:
        scale = C ** (-0.5)
    HD = H * D
    CK, SK, TK, MK = C // 128, S // 128, T // 128, HD // 128

    def nchunks(total):
        return [(i, min(512, total - i)) for i in range(0, total, 512)]

    nc = bacc.Bacc("TRN2", target_bir_lowering=False, debug=False,
                   num_devices=N_CORES)
    x = nc.dram_tensor("x", [T, C], F32R, kind="ExternalInput")
    y = nc.dram_tensor("y", [S, C], F32R, kind="ExternalInput")
    wq = nc.dram_tensor("wq", [C, HD], F32R, kind="ExternalInput")
    wk = nc.dram_tensor("wk", [C, HD], F32R, kind="ExternalInput")
    wv = nc.dram_tensor("wv", [C, HD], F32R, kind="ExternalInput")
    wot = nc.dram_tensor("wot", [HD, C], F32R, kind="ExternalInput")
    bo = nc.dram_tensor("bo", [1, C], F32R, kind="ExternalInput")
    ident_dr = nc.dram_tensor("ident", [128, 128], F32R, kind="ExternalInput")
    ones_dr = nc.dram_tensor("onesc", [128, 128], F32R, kind="ExternalInput")
    o = nc.dram_tensor("o", [T, C], F32, kind="ExternalOutput")

    def wpool_alloc(stack, poolname, rows, cols, side=None):
        """Open a bufs=1 pool holding `rows//128` weight tiles [128, cols]."""
        p = stack.enter_context(tc.tile_pool(name=poolname, bufs=1, side=side))
        return [p.tile([128, cols], F32R, tag=f"{poolname}{k}",
                       name=f"{poolname}{k}") for k in range(rows // 128)]

    def wdma(tiles, dram):
        for k, t in enumerate(tiles):
            nc.sync.dma_start(out=t, in_=dram[k * 128:(k + 1) * 128, :])

    def wpool_load(stack, poolname, dram, rows, cols, side=None):
        tiles = wpool_alloc(stack, poolname, rows, cols, side=side)
        wdma(tiles, dram)
        return tiles

    # SBUF pools are a two-sided stack allocator: open/close must be LIFO
    # per side.  Left: pers > yt > (wq > xt > stage), wk, (at, et).
    # Right: wv, then (wot > dpool).
    with tile.TileContext(nc) as tc, ExitStack() as ctx:
        pers = ctx.enter_context(tc.tile_pool(name="pers", bufs=1))

        ident = pers.tile([128, 128], F32R, tag="ident")
        nc.sync.dma_start(out=ident[:], in_=ident_dr[:, :])
        ones_sb = pers.tile([1, 128], F32R, tag="ones")
        nc.sync.dma_start(out=ones_sb[:], in_=ones_dr[0:1, :])

        qt = [pers.tile([128, T], BF16, tag=f"qt{m}", name=f"qt{m}")
              for m in range(MK)]
        kt0 = [pers.tile([128, S], BF16, tag=f"kt0_{m}", name=f"kt0_{m}")
               for m in range(MK)]
        kt1 = [pers.tile([128, S], BF16, tag=f"kt1_{m}", name=f"kt1_{m}")
               for m in range(MK)]
        v_sb = [pers.tile([128, H, 65], BF16, tag=f"v{s}", name=f"v{s}")
                for s in range(SK)]

        # explicit phase-scoped stacks (closed LIFO per side)
        s_wq, s_wk, s_wv, s_wot = ExitStack(), ExitStack(), ExitStack(), ExitStack()
        s_stage, s_xt, s_yt = ExitStack(), ExitStack(), ExitStack()
        s_psa, s_psb, s_psc, s_pss = (ExitStack(), ExitStack(),
                                      ExitStack(), ExitStack())
        try:
            # ---- phase A: load + transpose x, y ---------------------------
            p_yt = s_yt.enter_context(tc.tile_pool(name="ytp", bufs=1))
            yt = [p_yt.tile([128, S], F32R, tag=f"yt{k}", name=f"ytt{k}")
                  for k in range(CK)]

            wq_sb = wpool_alloc(s_wq, "wq", C, HD)

            p_xt = s_xt.enter_context(tc.tile_pool(name="xtp", bufs=1))
            xt = [p_xt.tile([128, T], F32R, tag=f"xt{k}", name=f"xtt{k}")
                  for k in range(CK)]

            p_stage = s_stage.enter_context(tc.tile_pool(name="stage", bufs=4))
            ps_t = s_psa.enter_context(
                tc.tile_pool(name="ps_t", bufs=4, space="PSUM"))

            def load_transpose(dram, n_tiles, i0, dst_tiles, dst_off):
                """Load up to 4 row-tiles of one C-half of `dram`; transpose
                the 4 C-chunks of that half into dst tiles."""
                n = min(4, n_tiles - i0)
                for coff, csz in nchunks(C):
                    stages = []
                    for i in range(n):
                        st = p_stage.tile([128, csz], F32R, tag="stage",
                                          name="stage")
                        nc.sync.dma_start(
                            out=st,
                            in_=dram[(i0 + i) * 128:(i0 + i + 1) * 128,
                                     coff:coff + csz])
                        stages.append(st)
                    for q2 in range(csz // 128):
                        ck = coff // 128 + q2
                        pst = ps_t.tile([128, n * 128], F32R, tag="pst",
                                        name="pst")
                        for q in range(n):
                            # pure permutation, no accumulation happens
                            with nc.allow_low_precision(reason="transpose"):
                                nc.tensor.transpose(
                                    pst[:, q * 128:(q + 1) * 128],
                                    stages[q][:, q2 * 128:(q2 + 1) * 128],
                                    ident[:])
                        nc.vector.tensor_copy(
                            dst_tiles[ck][:, dst_off:dst_off + n * 128],
                            pst[:])

            for i0 in range(0, TK, 4):
                load_transpose(x, TK, i0, xt, i0 * 128)
            wdma(wq_sb, wq)   # after x stages: x transposes start promptly
            for i0 in range(0, SK, 4):
                load_transpose(y, SK, i0, yt, i0 * 128)
            s_stage.close()
            s_psa.close()

            # ---- phase B: qT, kT ------------------------------------------
            ps_s = s_pss.enter_context(
                tc.tile_pool(name="ps_s", bufs=4, space="PSUM"))
            ps_qk = s_psb.enter_context(
                tc.tile_pool(name="ps_qk", bufs=3, space="PSUM"))
            for m in range(MK):
                psq = ps_qk.tile([128, T], F32, tag="psqk", name="psq")
                for k in range(CK):
                    nc.tensor.matmul(
                        psq[:], wq_sb[k][:, m * 128:(m + 1) * 128],
                        xt[k][:], start=(k == 0), stop=(k == CK - 1))
                nc.vector.tensor_copy(qt[m][:], psq[:])
            s_xt.close()
            s_wq.close()

            # et pool opens on the right BEFORE wv so scores/exp can start
            # in phase B while ACT is otherwise idle (LIFO: et outlives wv).
            p_et = ctx.enter_context(
                tc.tile_pool(name="etpool", bufs=et_bufs, side="right"))
            ets = {}

            def emit_scores(j):
                """scoresT + exp for head pair j (full-grid K=128 matmuls
                against zero-padded kt0/kt1)."""
                et0, et1 = [], []
                for s in range(SK):
                    pss0 = ps_s.tile([128, T], F32, tag="pss", name="pss")
                    nc.tensor.matmul(
                        pss0[:], kt0[j][:, s * 128:(s + 1) * 128],
                        qt[j][:, :], start=True, stop=True)
                    pss1 = ps_s.tile([128, T], F32, tag="pss", name="pss")
                    nc.tensor.matmul(
                        pss1[:], kt1[j][:, s * 128:(s + 1) * 128],
                        qt[j][:, :], start=True, stop=True)
                    e0 = p_et.tile([128, T], BF16, tag="et", name="et")
                    nc.scalar.activation(out=e0[:], in_=pss0[:], func=EXP,
                                         scale=float(scale))
                    e1 = p_et.tile([128, T], BF16, tag="et", name="et")
                    nc.scalar.activation(out=e1[:], in_=pss1[:], func=EXP,
                                         scale=float(scale))
                    et0.append(e0)
                    et1.append(e1)
                ets[j] = (et0, et1)

            wk_sb = wpool_load(s_wk, "wk", wk, C, HD)
            wv_sb = wpool_load(s_wv, "wv", wv, C, HD, side="right")

            for m in range(MK):
                nc.vector.memset(kt0[m][64:128, :], 0.0)
                nc.vector.memset(kt1[m][0:64, :], 0.0)
                for off, sz in nchunks(S):
                    psk = ps_qk.tile([128, sz], F32, tag="psqk", name="psk")
                    for k in range(CK):
                        nc.tensor.matmul(
                            psk[:], wk_sb[k][:, m * 128:(m + 1) * 128],
                            yt[k][:, off:off + sz],
                            start=(k == 0), stop=(k == CK - 1))
                    nc.vector.tensor_copy(kt0[m][0:64, off:off + sz],
                                          psk[0:64, :])
                    nc.vector.tensor_copy(kt1[m][64:128, off:off + sz],
                                          psk[64:128, :])
                emit_scores(m)
            s_wk.close()
            s_psb.close()

            # ---- phase C: v natural + ones column -------------------------
            ps_v = s_psc.enter_context(
                tc.tile_pool(name="ps_v", bufs=3, space="PSUM"))
            for s in range(SK):
                nc.vector.memset(v_sb[s][:, :, 64:65], 1.0)
                for off, sz in nchunks(HD):
                    psv = ps_v.tile([128, sz], F32, tag="psv", name="psv")
                    for k in range(CK):
                        nc.tensor.matmul(
                            psv[:], yt[k][:, s * 128:(s + 1) * 128],
                            wv_sb[k][:, off:off + sz],
                            start=(k == 0), stop=(k == CK - 1))
                    h0 = off // 64
                    nc.vector.tensor_copy(
                        v_sb[s][:, h0:h0 + sz // 64, 0:64],
                        psv[:].rearrange("p (h d) -> p h d", d=64))
            s_wv.close()
            s_yt.close()
            s_psc.close()

            # ---- phase D: attention per head ------------------------------
            wot_sb = wpool_load(s_wot, "wot", wot, HD, C, side="right")

            with ExitStack() as ctx_d:
                p_d = ctx_d.enter_context(
                    tc.tile_pool(name="dpool", bufs=2, side="right"))
                bo_sb = p_d.tile([1, C], F32R, tag="bo", bufs=1, name="bo_sb")
                nc.sync.dma_start(out=bo_sb, in_=bo[:, :])

                p_at = ctx_d.enter_context(tc.tile_pool(name="atpool", bufs=1))
                at = [p_at.tile([128, T], F32R, tag=f"at{m}", name=f"at{m}")
                      for m in range(MK)]
                ps_av = ctx_d.enter_context(
                    tc.tile_pool(name="ps_av", bufs=2, space="PSUM"))

                # 4 heads' denominators parked at partition bases 0/32/64/96
                # of one tile -> a single batched DVE reciprocal (DVE recip
                # cost is free-size-bound, so 1 op for 4 heads).
                group = []          # [(head, avs_tile)] awaiting normalize
                den4 = [None]

                def flush_group(last=False):
                    if not group:
                        return
                    rec4 = den4[0]
                    nc.vector.reciprocal(rec4[:], rec4[:])   # in place
                    for i, (h, avs) in enumerate(group):
                        r0 = p_d.tile([1, T], F32R, tag="r0", name="r0")
                        nc.vector.tensor_copy(
                            r0[:], rec4[32 * i:32 * i + 1, :])
                        psb = ps_av.tile([64, T], F32, tag="psav",
                                         name="psbt")
                        nc.tensor.matmul(psb[:], ones_sb[0:1, 0:64],
                                         r0[:], start=True, stop=True)
                        nc.vector.tensor_mul(
                            at[h // 2][(h % 2) * 64:(h % 2) * 64 + 64, :],
                            avs[:], psb[:])
                    group.clear()
                    den4[0] = None

                def stage_av(h, psav):
                    """Move AV rows out of PSUM right away; park denom row."""
                    if den4[0] is None:
                        den4[0] = p_d.tile([97, T], F32, tag="den4",
                                           name="den4")
                        nc.vector.memset(den4[0][:], 1.0)
                    avs = p_d.tile([64, T], F32, tag="avs", bufs=6, name="avs")
                    nc.vector.tensor_copy(avs[:], psav[0:64, :])
                    b = 32 * len(group)
                    nc.vector.tensor_copy(den4[0][b:b + 1, :], psav[64:65, :])
                    group.append((h, avs))

                def emit_av(j, et0, et1):
                    """AV for pair j — 16 back-to-back full-row matmuls."""
                    psav0 = ps_av.tile([65, T], F32, tag="psav", name="psav")
                    for s in range(SK):
                        nc.tensor.matmul(psav0[:], v_sb[s][:, 2 * j, 0:65],
                                         et0[s][:],
                                         start=(s == 0), stop=(s == SK - 1))
                    stage_av(2 * j, psav0)
                    psav1 = ps_av.tile([65, T], F32, tag="psav", name="psav")
                    for s in range(SK):
                        nc.tensor.matmul(psav1[:], v_sb[s][:, 2 * j + 1, 0:65],
                                         et1[s][:],
                                         start=(s == 0), stop=(s == SK - 1))
                    stage_av(2 * j + 1, psav1)

                for j in range(H // 2):
                    if len(group) == 4:
                        flush_group()
                    emit_av(j, *ets[j])
                flush_group(last=True)

                # ---- phase E: output projection + bias --------------------
                ps_o = ctx_d.enter_context(
                    tc.tile_pool(name="ps_o", bufs=2, space="PSUM"))
                for t_ in range(TK):
                    o_sb = p_d.tile([128, C], F32, tag="osb", name="o_sb")
                    for off, sz in nchunks(C):
                        pso = ps_o.tile([128, sz], F32, tag="pso", name="pso")
                        for mk in range(MK):
                            nc.tensor.matmul(
                                pso[:], at[mk][:, t_ * 128:(t_ + 1) * 128],
                                wot_sb[mk][:, off:off + sz],
                                start=(mk == 0), stop=False)
                        nc.tensor.matmul(pso[:], ones_sb[0:1, 0:128],
                                         bo_sb[0:1, off:off + sz],
                                         start=False, stop=True)
                        nc.scalar.activation(out=o_sb[:, off:off + sz],
                                             in_=pso[:], func=COPY)
                    nc.sync.dma_start(out=o[t_ * 128:(t_ + 1) * 128, :],
                                      in_=o_sb[:])
            s_wot.close()
        finally:
            for s in (s_stage, s_xt, s_wq, s_wk, s_psa, s_psb, s_psc,
                      s_wv, s_yt, s_wot, s_pss):
                s.close()

    nc.compile()
    return nc


# ---------------------------------------------------------------------------
# Host-side wrapper
# ---------------------------------------------------------------------------

_NC_CACHE = {}
_IDENT = np.eye(128, dtype=np.float32)
_ONESC = np.ones((128, 128), dtype=np.float32)


def _get_nc():
    key = "full"
    if key not in _NC_CACHE:
        _NC_CACHE[key] = build_attention_nc(
            T=T_FULL * B_FULL // N_CORES, S=S_FULL, C=C_FULL, H=H_FULL,
            D=D_FULL, scale=C_FULL ** (-0.5))
    return _NC_CACHE[key]


def make_in_maps(x, y_enc, Wq, Wk, Wv, Wo, bo):
    """Shard full inputs into the 8 per-core input maps."""
    x = np.asarray(x, dtype=np.float32)
    y_enc = np.asarray(y_enc, dtype=np.float32)
    Wq = np.asarray(Wq, dtype=np.float32)
    Wk = np.asarray(Wk, dtype=np.float32)
    Wv = np.asarray(Wv, dtype=np.float32)
    Wo = np.asarray(Wo, dtype=np.float32)
    bo = np.asarray(bo, dtype=np.float32)

    C = Wq.shape[1]
    HD = Wq.shape[0] * Wq.shape[2]
    wq_p = np.ascontiguousarray(Wq.transpose(1, 0, 2).reshape(C, HD))
    wk_p = np.ascontiguousarray(Wk.transpose(1, 0, 2).reshape(C, HD))
    wv_p = np.ascontiguousarray(Wv.transpose(1, 0, 2).reshape(C, HD))
    wot = np.ascontiguousarray(Wo.T)
    bo2 = np.ascontiguousarray(bo.reshape(1, -1))

    T = x.shape[1] * x.shape[0] // N_CORES  # rows per core
    in_maps = []
    for core in range(N_CORES):
        b, half = divmod(core, N_CORES // x.shape[0])
        in_maps.append({
            "x": np.ascontiguousarray(x[b, half * T:(half + 1) * T]),
            "y": np.ascontiguousarray(y_enc[b]),
            "wq": wq_p, "wk": wk_p, "wv": wv_p, "wot": wot, "bo": bo2,
            "ident": _IDENT, "onesc": _ONESC,
        })
    return in_maps


def run(inputs, trace=False, trace_cores=None):
    """Compile + run on the 8 cores; returns (out, BassKernelResults)."""
    nc = _get_nc()
    in_maps = make_in_maps(**inputs)
    kw = {}
    if trace:
        kw = dict(trace=True,
                  trace_cores=trace_cores if trace_cores is not None else [0])
    res = run_bass_kernel_spmd(nc, in_maps, core_ids=list(range(N_CORES)), **kw)

    B, T_full, C = np.asarray(inputs["x"]).shape
    T = T_full * B // N_CORES
    out = np.empty((B, T_full, C), dtype=np.float32)
    for core in range(N_CORES):
        b, half = divmod(core, N_CORES // B)
        out[b, half * T:(half + 1) * T] = res.results[core]["o"]
    return out, res


def kernel(x, y_enc, Wq, Wk, Wv, Wo, bo):
    out, _ = run(dict(x=x, y_enc=y_enc, Wq=Wq, Wk=Wk, Wv=Wv, Wo=Wo, bo=bo))
    return out
